# revision 1
# baseline (speedup 1.0000x reference)
"""Bass/Trainium2 kernel for nn_Attention_28140625723842.

Multi-head attention (B=2, S=2048, D=1024, H=16, DH=64) with key-padding
mask, sharded over 8 NeuronCores as 2 batches x 4 head-groups (tensor
parallel over heads, data parallel over batch).

Per-core strategy:
  - Host passes transposed activations qT/kT/vT [D, *] (bf16) so the
    d-contraction sits on SBUF partitions; k/v token columns are gathered
    down to the unmasked set (padded to a multiple of 128) — masked keys
    contribute exactly 0 to softmax numerator and denominator, so the
    result is unchanged while scores/exp/PV work halves.
  - Projections produce QT/KT transposed [dh, tokens] (2 heads stacked per
    128 partitions) and V natural [tokens, 4*(dh+1)] with a ones column
    per head.
  - scoresT[k, q] = KT_tile.T @ QT; the pad-key mask becomes a
    per-partition bias fused into the ScalarE exp:
    expS = exp(scores/sqrt(D) + (-1e9)*(1-mask)).
  - P@V uses lhsT = [V_h | 1] so the softmax denominator (row sum) comes
    out as column dh of the matmul output; a PE transpose brings each
    [65, 128] block to [q, 65] layout, where DVE reciprocal +
    tensor_scalar_mul normalize straight into the fp32 output buffer.
  - All matmul operands are bf16 (PSUM accumulation fp32); output fp32.
"""

import numpy as np

B, S, D, H = 2, 2048, 1024, 16
DH = D // H            # 64 head dim
NCORES = 8
GROUPS = NCORES // B   # 4 head groups
HL = H // GROUPS       # 4 heads per core
GW = HL * DH           # 256 output columns per core

P = 128
ND = D // P            # 8 contraction tiles
NT = S // P            # 16 q token tiles
QB = 1024              # q block (one exp op width)
NQB = S // QB          # 2
CH = 512               # matmul free-dim chunk (one PSUM bank fp32)
NCH = QB // CH         # 2

COMPACT = True         # gather unmasked k/v tokens on host

_CACHE = {}


def _chunks(total, width):
    out = []
    o = 0
    while o < total:
        w = min(width, total - o)
        out.append((o, w))
        o += w
    return out


def _build_nc(nk, use_bias=True, debug=False, pv_inter=False, w_scalar=True):
    import concourse.bacc as bacc
    import concourse.mybir as mybir
    import concourse.tile as tile
    from concourse.masks import make_identity

    f32 = mybir.dt.float32
    bf16 = mybir.dt.bfloat16
    i32 = mybir.dt.int32
    Exp = mybir.ActivationFunctionType.Exp
    SCALE = float(1.0 / np.sqrt(np.float32(D)))
    NTK = nk // P          # k token tiles (compacted)

    nc = bacc.Bacc(None, target_bir_lowering=False)
    qt_d = nc.dram_tensor("qt", [D, S], bf16, kind="ExternalInput")
    kt_d = nc.dram_tensor("kt", [D, nk], bf16, kind="ExternalInput")
    vt_d = nc.dram_tensor("vt", [D, nk], bf16, kind="ExternalInput")
    wq_d = nc.dram_tensor("wq", [D, GW], bf16, kind="ExternalInput")
    wk_d = nc.dram_tensor("wk", [D, GW], bf16, kind="ExternalInput")
    wv_d = nc.dram_tensor("wv", [D, GW], bf16, kind="ExternalInput")
    bq_d = nc.dram_tensor("bq", [GW], bf16, kind="ExternalInput")
    bk_d = nc.dram_tensor("bk", [GW], bf16, kind="ExternalInput")
    bv_d = nc.dram_tensor("bv", [GW], bf16, kind="ExternalInput")
    mask_d = nc.dram_tensor("mask", [nk], i32, kind="ExternalInput")
    out_d = nc.dram_tensor("out", [S, GW], f32, kind="ExternalOutput")
    if debug:
        dbg_qt = nc.dram_tensor("dbg_qt", [P, HL // 2, S], bf16, kind="ExternalOutput")
        dbg_kt = nc.dram_tensor("dbg_kt", [P, HL // 2, nk], bf16, kind="ExternalOutput")
        dbg_v = nc.dram_tensor("dbg_v", [P, NTK, HL * (DH + 1)], bf16, kind="ExternalOutput")

    with tile.TileContext(nc) as tc:
        with (
            tc.tile_pool(name="consts", bufs=1) as consts,
            tc.tile_pool(name="persist", bufs=1) as persist,
            tc.tile_pool(name="wpool", bufs=3) as wpool,
            tc.tile_pool(name="xt", bufs=8) as xtp,
            tc.tile_pool(name="vx", bufs=ND) as vxp,
            tc.tile_pool(name="exps", bufs=2 * NTK + 2) as expp,
            tc.tile_pool(name="tmp", bufs=8) as tmpp,
            tc.tile_pool(name="rec", bufs=8) as recp,
        ):
            ident = consts.tile([P, P], f32, tag="ident")
            make_identity(nc, ident)
            ones = consts.tile([1, CH], bf16, tag="ones")
            nc.vector.memset(ones, 1.0)

            # mask[k] -> per-partition exp bias: (m - 1) * 1e9  (0 or -1e9)
            maski = consts.tile([P, NTK], i32, tag="maski")
            nc.scalar.dma_start(maski, mask_d.rearrange("(t p) -> p t", p=P))
            maskb = consts.tile([P, NTK], f32, tag="maskb")
            nc.vector.tensor_scalar(
                maskb, maski, -1.0, 1e9,
                mybir.AluOpType.add, mybir.AluOpType.mult,
            )

            brow = {}
            if use_bias:
                for nm, drm in (("q", bq_d), ("k", bk_d), ("v", bv_d)):
                    t = consts.tile([1, GW], bf16, tag=f"bias_{nm}")
                    nc.scalar.dma_start(t, drm[None, :])
                    brow[nm] = t

            QT = persist.tile([P, HL // 2, S], bf16, tag="QT")
            KT = persist.tile([P, HL // 2, nk], bf16, tag="KT")
            V = persist.tile([P, NTK, HL * (DH + 1)], bf16, tag="V")
            V4 = V.rearrange("p t (h e) -> p t h e", h=HL)
            out_sb = persist.tile([P, NT, GW], f32, tag="osb")

            for h in range(HL):
                nc.vector.memset(V4[:, :, h, DH], 1.0)

            xvt = []

            with tc.tile_pool(name="pps", bufs=8, space="PSUM") as pps:
                # ---- QT / KT projections: out[dh2, tok] accumulated over d ----
                for nm, xdr, wdr, bkey, OUT, width in (
                    ("q", qt_d, wq_d, "q", QT, S),
                    ("k", kt_d, wk_d, "k", KT, nk),
                ):
                    if nm == "k":
                        # V-projection activations on the scalar HWDGE ring:
                        # streams during late-Q/K compute without competing
                        # with the Q loads for HBM bandwidth.
                        for dt_ in range(ND):
                            t = vxp.tile([P, nk], bf16, tag="xvt",
                                         name=f"xvt_{dt_}")
                            nc.scalar.dma_start(t, vt_d[dt_ * P:(dt_ + 1) * P, :])
                            xvt.append(t)
                    w_sb = wpool.tile([P, ND, GW], bf16, tag="w")
                    wdr_blk = wdr.rearrange("(n p) w -> p n w", p=P)
                    chs = _chunks(width, CH)
                    pst = {}
                    weng = nc.scalar if w_scalar else nc.sync
                    for dt_ in range(ND):
                        weng.dma_start(w_sb[:, dt_, :], wdr_blk[:, dt_, :])
                        x_sb = xtp.tile([P, S], bf16, tag="xt",
                                        name=f"x_{nm}_{dt_}")
                        if dt_ == 0:
                            cuts = [0, min(CH, width), min(2 * CH, width), width]
                            for a, b in zip(cuts, cuts[1:]):
                                if b > a:
                                    nc.sync.dma_start(
                                        x_sb[:, a:b],
                                        xdr[dt_ * P:(dt_ + 1) * P, a:b])
                        else:
                            nc.sync.dma_start(x_sb[:, :width],
                                              xdr[dt_ * P:(dt_ + 1) * P, :])
                        for hp in range(HL // 2):
                            for ci, (co, cw) in enumerate(chs):
                                if dt_ == 0:
                                    pst[(hp, ci)] = pps.tile(
                                        [P, CH], f32, tag="pp",
                                        name=f"pp_{nm}_{hp}_{ci}")
                                nc.tensor.matmul(
                                    pst[(hp, ci)][:, :cw],
                                    lhsT=w_sb[:, dt_, hp * P:(hp + 1) * P],
                                    rhs=x_sb[:, co:co + cw],
                                    start=(dt_ == 0),
                                    stop=(not use_bias and dt_ == ND - 1),
                                )
                    for hp in range(HL // 2):
                        for ci, (co, cw) in enumerate(chs):
                            if use_bias:
                                nc.tensor.matmul(
                                    pst[(hp, ci)][:, :cw],
                                    lhsT=brow[bkey][:, hp * P:(hp + 1) * P],
                                    rhs=ones[:, :cw],
                                    start=False, stop=True,
                                )
                            nc.vector.tensor_copy(
                                out=OUT[:, hp, co:co + cw],
                                in_=pst[(hp, ci)][:, :cw],
                            )

                # ---- V projection: natural [tok, 4*dh] ----
                # tok-tile outer so each PSUM accumulation group owns a
                # whole bank (start=True clears has_written bank-wide).
                wv_sb = wpool.tile([P, ND, GW], bf16, tag="w")
                (nc.scalar if w_scalar else nc.sync).dma_start(wv_sb, wv_d.rearrange("(n p) w -> p n w", p=P))
                for tt in range(NTK):
                    vp_ps = pps.tile([P, GW], f32, tag="pp", name=f"ppv_{tt}")
                    for dt_ in range(ND):
                        nc.tensor.matmul(
                            vp_ps,
                            lhsT=xvt[dt_][:, tt * P:(tt + 1) * P],
                            rhs=wv_sb[:, dt_, :],
                            start=(dt_ == 0),
                            stop=(not use_bias and dt_ == ND - 1),
                        )
                    if use_bias:
                        nc.tensor.matmul(
                            vp_ps,
                            lhsT=ones[:, :P],
                            rhs=brow["v"],
                            start=False, stop=True,
                        )
                    nc.vector.tensor_copy(
                        out=V4[:, tt, :, :DH],
                        in_=vp_ps.rearrange("p (h e) -> p h e", h=HL),
                    )

            # ---- attention ----
            with (
                tc.tile_pool(name="pss", bufs=2, space="PSUM") as pss,
                tc.tile_pool(name="pspv", bufs=2, space="PSUM") as pspv,
                tc.tile_pool(name="pstr", bufs=2, space="PSUM") as pstr,
            ):
                out_blk = out_d.rearrange("(t p) w -> p t w", p=P)

                def emit_normalize(items):
                    # deferred tail of a block: PE transposes + DVE normalize
                    for pv_sb, hh, qq, c in items:
                        for q4 in range(CH // P):
                            tps = pstr.tile([P, DH + 1], f32, tag="tr")
                            nc.tensor.transpose(
                                tps,
                                pv_sb[:, q4 * P:(q4 + 1) * P],
                                ident[:DH + 1, :DH + 1],
                            )
                            rec = recp.tile([P, 1], f32, tag="rec")
                            nc.vector.reciprocal(rec, tps[:, DH:DH + 1])
                            tokt = qq * (QB // P) + c * (CH // P) + q4
                            nc.vector.tensor_scalar_mul(
                                out_sb[:, tokt, hh * DH:(hh + 1) * DH],
                                tps[:, :DH],
                                rec,
                            )
                        if hh == HL - 1:
                            t0 = qq * (QB // P) + c * (CH // P)
                            t1 = t0 + CH // P
                            nc.sync.dma_start(
                                out_blk[:, t0:t1, :], out_sb[:, t0:t1, :]
                            )

                pending = []
                for h in range(HL):
                    hp, ho = divmod(h, 2)
                    po = ho * DH  # partition offset within the stacked pair
                    for qb in range(NQB):
                        etiles = []
                        for kt_ in range(NTK):
                            sps = pss.tile([P, QB], f32, tag="s",
                                           name=f"s_{h}_{qb}_{kt_}")
                            for c in range(NCH):
                                nc.tensor.matmul(
                                    sps[:, c * CH:(c + 1) * CH],
                                    lhsT=KT[po:po + DH, hp, kt_ * P:(kt_ + 1) * P],
                                    rhs=QT[po:po + DH, hp,
                                           qb * QB + c * CH:qb * QB + (c + 1) * CH],
                                    start=True, stop=True,
                                )
                            e = expp.tile([P, QB], bf16, tag="e",
                                          name=f"e_{h}_{qb}_{kt_}")
                            nc.scalar.activation(
                                e, sps, Exp,
                                bias=maskb[:, kt_:kt_ + 1], scale=SCALE,
                            )
                            etiles.append(e)
                            if kt_ == 1 and pending:
                                emit_normalize(pending)
                                pending = []
                            if kt_ == 0:
                                pvt = [pspv.tile([DH + 1, CH], f32, tag="pv",
                                                 name=f"pv_{h}_{qb}_{c}")
                                       for c in range(NCH)]
                            kp = kt_ - 2
                            if pv_inter and kp >= 0:
                                for c in range(NCH):
                                    nc.tensor.matmul(
                                        pvt[c],
                                        lhsT=V[:, kp, h * (DH + 1):(h + 1) * (DH + 1)],
                                        rhs=etiles[kp][:, c * CH:(c + 1) * CH],
                                        start=(kp == 0), stop=False,
                                    )
                        for kp in (range(NTK - 2, NTK) if pv_inter
                                   else range(NTK)):
                            for c in range(NCH):
                                nc.tensor.matmul(
                                    pvt[c],
                                    lhsT=V[:, kp, h * (DH + 1):(h + 1) * (DH + 1)],
                                    rhs=etiles[kp][:, c * CH:(c + 1) * CH],
                                    start=(kp == 0), stop=(kp == NTK - 1),
                                )
                        for c in range(NCH):
                            pv_sb = tmpp.tile([DH + 1, CH], f32, tag="pvsb",
                                              name=f"pvsb_{h}_{qb}_{c}")
                            nc.vector.tensor_copy(out=pv_sb, in_=pvt[c])
                            pending.append((pv_sb, h, qb, c))
                emit_normalize(pending)

                if debug:
                    nc.sync.dma_start(dbg_qt[:], QT)
                    nc.sync.dma_start(dbg_kt[:], KT)
                    nc.sync.dma_start(dbg_v[:], V)
    nc.compile()
    return nc


def _get_nc(nk, use_bias=True, debug=False, pv_inter=False, w_scalar=True):
    key = (nk, use_bias, debug, pv_inter, w_scalar)
    if key not in _CACHE:
        _CACHE[key] = _build_nc(nk, use_bias=use_bias, debug=debug,
                                pv_inter=pv_inter, w_scalar=w_scalar)
    return _CACHE[key]


def _run(nc, in_maps, trace=False):
    from concourse.bass_utils import run_bass_kernel_spmd

    return run_bass_kernel_spmd(
        nc, in_maps, core_ids=list(range(NCORES)), trace=trace
    )


def _make_in_maps(q, k, v, mask, Wq, bq, Wk, bk, Wv, bv):
    import ml_dtypes

    bf16 = ml_dtypes.bfloat16
    q = np.asarray(q, np.float32)
    k = np.asarray(k, np.float32)
    v = np.asarray(v, np.float32)
    mask = np.asarray(mask, np.int32)
    Wq = np.asarray(Wq, np.float32).astype(bf16)
    Wk = np.asarray(Wk, np.float32).astype(bf16)
    Wv = np.asarray(Wv, np.float32).astype(bf16)
    bq = np.asarray(bq, np.float32).astype(bf16)
    bk = np.asarray(bk, np.float32).astype(bf16)
    bv = np.asarray(bv, np.float32).astype(bf16)

    use_bias = bool(
        np.any(np.asarray(bq, np.float32))
        or np.any(np.asarray(bk, np.float32))
        or np.any(np.asarray(bv, np.float32))
    )
    if COMPACT:
        idxs = [np.nonzero(mask[b])[0] for b in range(B)]
        neff = max(1, max(len(ix) for ix in idxs))
        nk = -(-neff // P) * P  # round up to multiple of 128
    else:
        idxs = [np.arange(S) for _ in range(B)]
        nk = S

    qT = [np.ascontiguousarray(q[b].T).astype(bf16) for b in range(B)]
    kT, vT, mk = [], [], []
    for b in range(B):
        ix = idxs[b]
        kc = np.zeros((D, nk), bf16)
        vc = np.zeros((D, nk), bf16)
        kc[:, :len(ix)] = k[b].T[:, ix].astype(bf16)
        vc[:, :len(ix)] = v[b].T[:, ix].astype(bf16)
        kT.append(kc)
        vT.append(vc)
        m = np.zeros((nk,), np.int32)
        if COMPACT:
            m[:len(ix)] = 1
        else:
            m[:] = mask[b]
        mk.append(m)

    in_maps = []
    for c in range(NCORES):
        b, g = divmod(c, GROUPS)
        cols = slice(g * GW, (g + 1) * GW)
        in_maps.append({
            "qt": qT[b],
            "kt": kT[b],
            "vt": vT[b],
            "wq": np.ascontiguousarray(Wq[:, cols]),
            "wk": np.ascontiguousarray(Wk[:, cols]),
            "wv": np.ascontiguousarray(Wv[:, cols]),
            "bq": np.ascontiguousarray(bq[cols]),
            "bk": np.ascontiguousarray(bk[cols]),
            "bv": np.ascontiguousarray(bv[cols]),
            "mask": mk[b],
        })
    return nk, use_bias, in_maps


def _assemble(results):
    out = np.empty((B, S, D), np.float32)
    for c in range(NCORES):
        b, g = divmod(c, GROUPS)
        out[b, :, g * GW:(g + 1) * GW] = results[c]["out"]
    return out


def kernel(q, k, v, mask, Wq, bq, Wk, bk, Wv, bv):
    nk, use_bias, in_maps = _make_in_maps(q, k, v, mask, Wq, bq, Wk, bk, Wv, bv)
    res = _run(_get_nc(nk, use_bias), in_maps, trace=False)
    return _assemble(res.results)


def _install_ntff_hook():
    """The image's antenv stub lacks axon_hooks; synthesize it and register
    the ctypes NTFF hook that trn_agent_boot would have installed."""
    import sys
    import types

    import antenv

    if "antenv.axon_hooks" in sys.modules:
        return
    mod = types.ModuleType("antenv.axon_hooks")
    state = {"hook": None}
    mod.set_axon_ntff_profile_hook = lambda h: state.__setitem__("hook", h)
    mod.get_axon_ntff_profile_hook = lambda: state["hook"]
    sys.modules["antenv.axon_hooks"] = mod
    antenv.axon_hooks = mod
    try:
        from trn_agent_boot.trn_boot import _ntff_profile_via_ctypes

        mod.set_axon_ntff_profile_hook(
            _ntff_profile_via_ctypes("/opt/axon/libaxon_pjrt.so")
        )
    except Exception as e:
        print(f"ntff hook registration failed: {e}")


def kernel_traced(q, k, v, mask, Wq, bq, Wk, bk, Wv, bv):
    """Same as kernel() but also returns (output, exec_time_ns)."""
    _install_ntff_hook()
    nk, use_bias, in_maps = _make_in_maps(q, k, v, mask, Wq, bq, Wk, bk, Wv, bv)
    res = _run(_get_nc(nk, use_bias), in_maps, trace=True)
    return _assemble(res.results), res.exec_time_ns



# revision 2
# speedup vs baseline: 1.0372x; 1.0372x over previous
"""Bass/Trainium2 kernel v2 for nn_Attention_28140625723842.

MHA (B=2, S=2048, D=1024, H=16, DH=64) with key-padding mask, 8 cores as
2 batches x 4 head-groups (4 heads/core). Key techniques vs v1:

  - Q/K projections as fp8 DoubleRow matmuls (contraction 2x128 per
    instruction = 2x PE throughput); weights prescaled x64 on host, the
    1/64 folded into the PSUM->SBUF copy.
  - Score matmuls (K=dh=64) issued as row-group pairs via
    tile_position (0,0)/(64,0): two heads' matmuls run concurrently on
    the PE array halves (measured 2x).
  - Key compaction (masked keys dropped host-side) means no mask bias at
    all: padded key rows are killed via zeroed V rows + zeroed ones
    column (denominator), so exp needs no per-partition bias.
  - exp split across engines: ScalarE real exp + DVE/Pool Schraudolph
    (tensor_scalar fp32->int16 whose bits are the bf16 exp approx).
  - PV with V|ones stationary (M=65): denominator = column 64; PE
    transposes (bf16, batched 4/bank) -> [q, dh]; DVE reciprocal +
    broadcast multiply into fp32 out staging.
"""

import numpy as np

B, S, D, H = 2, 2048, 1024, 16
DH = D // H
NCORES = 8
GROUPS = NCORES // B
HL = H // GROUPS       # 4 heads per core
GW = HL * DH           # 256 out cols per core

P = 128
QB = 1024              # q block
NQB = S // QB          # 2
CH = 512
SCALE = float(1.0 / np.sqrt(np.float32(D)))
WSCL = 64.0            # host prescale on Wq/Wk (fp8 range)
SCHR_A = float(128.0 / np.log(2.0))
SCHR_B = 16253.0

_CACHE = {}


def _chunks(total, width):
    out, o = [], 0
    while o < total:
        w = min(width, total - o)
        out.append((o, w))
        o += w
    return out


def _build_nc(nk, use_bias=False):
    import concourse.bacc as bacc
    import concourse.mybir as mybir
    import concourse.tile as tile
    from concourse.masks import make_identity

    f32 = mybir.dt.float32
    bf16 = mybir.dt.bfloat16
    i16 = mybir.dt.int16
    fp8 = mybir.dt.float8e4
    Exp = mybir.ActivationFunctionType.Exp
    Copy = mybir.ActivationFunctionType.Copy
    DR = mybir.MatmulPerfMode.DoubleRow
    NTK = nk // P

    nc = bacc.Bacc(None, target_bir_lowering=False)
    qt8_d = nc.dram_tensor("qt8", [P, 8, S], fp8, kind="ExternalInput")
    kt8_d = nc.dram_tensor("kt8", [P, 8, nk], fp8, kind="ExternalInput")
    vt_d = nc.dram_tensor("vt", [P, nk, 8], bf16, kind="ExternalInput")
    wq8_d = nc.dram_tensor("wq8", [P, 8, GW], fp8, kind="ExternalInput")
    wk8_d = nc.dram_tensor("wk8", [P, 8, GW], fp8, kind="ExternalInput")
    wv_d = nc.dram_tensor("wv", [P, 8, GW], bf16, kind="ExternalInput")
    km_d = nc.dram_tensor("km", [P, NTK], bf16, kind="ExternalInput")
    bq_d = nc.dram_tensor("bq", [GW], bf16, kind="ExternalInput")
    bk_d = nc.dram_tensor("bk", [GW], bf16, kind="ExternalInput")
    bv_d = nc.dram_tensor("bv", [GW], bf16, kind="ExternalInput")
    out_d = nc.dram_tensor("out", [S // P, P, GW], bf16, kind="ExternalOutput")

    with tile.TileContext(nc) as tc:
        with (
            tc.tile_pool(name="consts", bufs=1) as consts,
            tc.tile_pool(name="persist", bufs=1) as persist,
            tc.tile_pool(name="ebuf", bufs=8) as ebp,
            tc.tile_pool(name="pvsb", bufs=4) as pvsbp,
            tc.tile_pool(name="recb", bufs=4) as recp,
        ):
            # ---- persistent SBUF ----
            qt8 = persist.tile([P, 8, S], fp8, tag="qt8")
            kt8 = persist.tile([P, 8, nk], fp8, tag="kt8")
            vt = persist.tile([P, nk, 8], bf16, tag="vt")
            wq8 = persist.tile([P, 8, GW], fp8, tag="wq8")
            wk8 = persist.tile([P, 8, GW], fp8, tag="wk8")
            wv = persist.tile([P, 8, GW], bf16, tag="wv")
            km = persist.tile([P, NTK], bf16, tag="km")
            QT = persist.tile([P, 2, S], bf16, tag="QT")
            KT = persist.tile([P, 2, nk], bf16, tag="KT")
            QT2 = persist.tile([P, HL, S], bf16, tag="QT2")
            KT2 = persist.tile([P, HL, nk], bf16, tag="KT2")
            V4 = persist.tile([P, NTK, HL, DH + 1], bf16, tag="V4")
            out_sb = persist.tile([P, S // P, GW], bf16, tag="osb")

            identb = consts.tile([P, P], bf16, tag="identb")
            ident = consts.tile([P, P], f32, tag="ident")
            make_identity(nc, ident)
            nc.vector.tensor_copy(out=identb, in_=ident)
            ones = consts.tile([1, P], bf16, tag="ones")
            nc.vector.memset(ones, 1.0)

            # weights + k/v activations on the scalar HWDGE ring,
            # q + mask on sync, so the q path gets its own bandwidth.
            nc.scalar.dma_start(wq8, wq8_d[:, :, :])
            nc.scalar.dma_start(wk8, wk8_d[:, :, :])
            nc.scalar.dma_start(wv, wv_d[:, :, :])
            nc.sync.dma_start(km, km_d[:, :])
            vt_chunks = list(_chunks(nk, 384))
            for i, (co, cw) in enumerate(_chunks(S, 256)):
                nc.sync.dma_start(qt8[:, :, co:co + cw],
                                  qt8_d[:, :, co:co + cw])
                if i >= 5 and vt_chunks:
                    vo, vw = vt_chunks.pop(0)
                    nc.sync.dma_start(vt[:, vo:vo + vw, :],
                                      vt_d[:, vo:vo + vw, :])
            for co, cw in _chunks(nk, 384):
                nc.scalar.dma_start(kt8[:, :, co:co + cw],
                                    kt8_d[:, :, co:co + cw])
            for vo, vw in vt_chunks:
                nc.sync.dma_start(vt[:, vo:vo + vw, :],
                                  vt_d[:, vo:vo + vw, :])

            brow = {}
            if use_bias:
                for nm, drm in (("q", bq_d), ("k", bk_d), ("v", bv_d)):
                    t = consts.tile([1, GW], bf16, tag=f"bias_{nm}")
                    nc.scalar.dma_start(t, drm[None, :])
                    brow[nm] = t

            with tc.tile_pool(name="pps", bufs=4, space="PSUM") as pps:
                # ---- Q/K projections via fp8 DoubleRow ----
                # chunk-outer so PE work tracks DMA arrival; Q and K
                # interleaved (they stream on different rings)
                qk_work = []
                for nm, x8, w8, OUT, width in (
                    ("q", qt8, wq8, QT, S),
                    ("k", kt8, wk8, KT, nk),
                ):
                    chs = ([(0, 256), (256, 256)] + _chunks(S - 512, CH)
                           if nm == "q" else _chunks(width, CH))
                    if nm == "q":
                        chs = [(0, 256), (256, 256)] + [
                            (o + 512, w) for o, w in _chunks(S - 512, CH)]
                    for ci, (co, cw) in enumerate(chs):
                        for hp in range(2):
                            qk_work.append((nm, x8, w8, OUT, hp, ci, co, cw))
                qk_work.sort(key=lambda w: (w[6] + (256 if w[0] == "k" else 0),
                                            w[0], w[4]))
                for nm, x8, w8, OUT, hp, ci, co, cw in qk_work:
                    if True:
                        if True:
                            ps = pps.tile([P, CH], f32, tag="pp",
                                          name=f"pp_{nm}_{hp}_{ci}")
                            for u in range(4):
                                nc.tensor.matmul(
                                    ps[:, :cw],
                                    lhsT=w8[:, 2 * u:2 * u + 2,
                                            hp * P:(hp + 1) * P],
                                    rhs=x8[:, 2 * u:2 * u + 2,
                                           co:co + cw],
                                    start=(u == 0),
                                    stop=(u == 3 and not use_bias),
                                    perf_mode=DR,
                                )
                            if use_bias:
                                nc.tensor.matmul(
                                    ps[:, :cw],
                                    lhsT=brow[nm][:, hp * P:(hp + 1) * P],
                                    rhs=ones[:, :cw],
                                    start=False, stop=True,
                                )
                            if ci % 2 == 0:
                                nc.vector.tensor_scalar_mul(
                                    OUT[:, hp, co:co + cw], ps[:, :cw],
                                    1.0 / WSCL)
                            else:
                                nc.scalar.activation(
                                    OUT[:, hp, co:co + cw], ps[:, :cw],
                                    Copy, scale=1.0 / WSCL)

                # ---- V projection (bf16, exact) ----
                for tt in range(NTK):
                    ps = pps.tile([P, GW], f32, tag="pp", name=f"ppv_{tt}")
                    for dt_ in range(8):
                        nc.tensor.matmul(
                            ps,
                            lhsT=vt[:, tt * P:(tt + 1) * P, dt_],
                            rhs=wv[:, dt_, :],
                            start=(dt_ == 0),
                            stop=(dt_ == 7 and not use_bias),
                        )
                    if use_bias:
                        nc.tensor.matmul(
                            ps, lhsT=ones[:1, :P], rhs=brow["v"],
                            start=False, stop=True,
                        )
                    nc.vector.tensor_copy(
                        out=V4[:, tt, :, :DH],
                        in_=ps.rearrange("p (h e) -> p h e", h=HL),
                    )
                # duplicate each head's projected Q/K rows into both
                # partition halves so chunk pairs can run row-tiled
                for h in range(HL):
                    hp, ho = divmod(h, 2)
                    so = ho * DH
                    for half, eng in ((0, nc.sync), (64, nc.scalar)):
                        eng.dma_start(
                            QT2[half:half + DH, h, :],
                            QT[so:so + DH, hp, :])
                        eng.dma_start(
                            KT2[half:half + DH, h, :],
                            KT[so:so + DH, hp, :])

                for h in range(HL):
                    # ones column = key-mask (0 on padded rows) so padded
                    # keys drop out of numerator and denominator alike
                    nc.gpsimd.tensor_copy(out=V4[:, :, h, DH], in_=km)

            # ---- attention ----
            with (
                tc.tile_pool(name="pss", bufs=3, space="PSUM") as pss,
                tc.tile_pool(name="pspv", bufs=2, space="PSUM") as pspv,
            ):
                out_blk = out_d.rearrange("t p w -> p t w")
                pending = None

                def emit_tail(hx, qbx, pvts):
                    # tail of a finished block: copies were already issued;
                    # transposes + reciprocal + normalize + out DMA
                    for c in range(2):
                        pv_sb, _ = pvts[c]
                        tr = pss.tile([P, 4, DH + 1], f32, tag="s",
                                      name=f"tr_{qbx}_{hx}_{c}")
                        for q4 in range(4):
                            nc.tensor.transpose(
                                tr[:, q4, :],
                                pv_sb[:, q4 * P:(q4 + 1) * P],
                                ident[:DH + 1, :DH + 1])
                        rec = recp.tile([P, 4], f32, tag="rec")
                        nc.vector.reciprocal(rec, tr[:, :, DH])
                        t0 = qbx * (QB // P) + c * 4
                        nc.vector.tensor_tensor(
                            out=out_sb[:, t0:t0 + 4,
                                       hx * DH:(hx + 1) * DH],
                            in0=tr[:, :, :DH],
                            in1=rec[:, :, None].broadcast_to([P, 4, DH]),
                            op=mybir.AluOpType.mult)
                    if hx == HL - 1:
                        t0 = qbx * (QB // P)
                        nc.sync.dma_start(
                            out_blk[:, t0:t0 + QB // P, :],
                            out_sb[:, t0:t0 + QB // P, :])

                for qb in range(NQB):
                    for h in range(HL):
                        e_tiles = {}
                        pvt = [pspv.tile([P, CH], f32, tag="pv",
                                         name=f"pv_{qb}_{h}_{c}")
                               for c in range(2)]

                        def pv_mms(kp, last):
                            for c in range(2):
                                nc.tensor.matmul(
                                    pvt[c][:DH + 1, :],
                                    lhsT=V4[:, kp, h, :],
                                    rhs=e_tiles[kp][:, c * CH:(c + 1) * CH],
                                    start=(kp == 0),
                                    stop=last,
                                )

                        def scores_exp(kt_):
                            e_tiles[kt_] = ebp.tile(
                                [P, QB], bf16, tag="e",
                                name=f"e_{qb}_{h}_{kt_}")
                            sps = pss.tile([P, QB], f32, tag="s",
                                           name=f"s_{qb}_{h}_{kt_}")
                            for c in range(2):
                                po = c * 64
                                nc.tensor.matmul(
                                    sps[:, c * CH:(c + 1) * CH],
                                    lhsT=KT2[po:po + DH, h,
                                             kt_ * P:(kt_ + 1) * P],
                                    rhs=QT2[po:po + DH, h,
                                            qb * QB + c * CH:
                                            qb * QB + (c + 1) * CH],
                                    start=True, stop=True,
                                    tile_position=(po, 0),
                                )
                            if kt_ in (0, 2, 4, 6, 7):
                                nc.scalar.activation(e_tiles[kt_], sps,
                                                     Exp, scale=SCALE)
                            else:
                                nc.vector.tensor_scalar(
                                    e_tiles[kt_].bitcast(i16), sps,
                                    SCHR_A * SCALE, SCHR_B,
                                    mybir.AluOpType.mult,
                                    mybir.AluOpType.add)

                        # kt pairs: scores for two kts back-to-back, then
                        # the lag-2 PV pair (fewer half/full array switches)
                        scores_exp(0)
                        scores_exp(1)
                        for kt_ in range(2, NTK, 2):
                            scores_exp(kt_)
                            if kt_ + 1 < NTK:
                                scores_exp(kt_ + 1)
                            if kt_ == 2 and pending is not None:
                                emit_tail(*pending)
                                pending = None
                            pv_mms(kt_ - 2, False)
                            if kt_ + 1 < NTK:
                                pv_mms(kt_ - 1, False)
                        pv_mms(NTK - 2, False)
                        pv_mms(NTK - 1, True)
                        pad = pss.tile([P, 4], f32, tag="s",
                                       name=f"pad_{qb}_{h}")
                        nc.vector.memset(pad, 0.0)
                        pvts = []
                        for c in range(2):
                            pv_sb = pvsbp.tile([DH + 1, CH], f32,
                                               tag="pvsb")
                            nc.scalar.activation(
                                pv_sb, pvt[c][:DH + 1, :], Copy)
                            pvts.append((pv_sb, c))
                        pending = (h, qb, pvts)
                emit_tail(*pending)
    nc.compile()
    return nc


def _get_nc(nk, use_bias=False):
    key = (nk, use_bias)
    if key not in _CACHE:
        _CACHE[key] = _build_nc(nk, use_bias=use_bias)
    return _CACHE[key]


def _make_in_maps(q, k, v, mask, Wq, bq, Wk, bk, Wv, bv):
    import ml_dtypes

    bf16 = ml_dtypes.bfloat16
    fp8 = ml_dtypes.float8_e4m3
    q = np.asarray(q, np.float32)
    k = np.asarray(k, np.float32)
    v = np.asarray(v, np.float32)
    mask = np.asarray(mask, np.int32)
    Wq = np.asarray(Wq, np.float32)
    Wk = np.asarray(Wk, np.float32)
    Wv = np.asarray(Wv, np.float32)

    use_bias = bool(
        np.any(np.asarray(bq, np.float32))
        or np.any(np.asarray(bk, np.float32))
        or np.any(np.asarray(bv, np.float32))
    )

    idxs = [np.nonzero(mask[b])[0] for b in range(B)]
    neff = max(1, max(len(ix) for ix in idxs))
    nk = -(-neff // P) * P

    def tile8(xT, width):
        # [D, width] -> [128, 8, width] (weights layout)
        return np.ascontiguousarray(
            xT.reshape(8, P, width).transpose(1, 0, 2))

    def tile8t(xT, width):
        # [D, width] -> [128, width, 8] (tok-major activations)
        return np.ascontiguousarray(
            xT.reshape(8, P, width).transpose(1, 2, 0))

    qt8 = [tile8((q[b].T * 1.0).astype(fp8), S) for b in range(B)]
    kt8, vt, kmv = [], [], []
    for b in range(B):
        ix = idxs[b]
        kc = np.zeros((D, nk), np.float32)
        vc = np.zeros((D, nk), np.float32)
        kc[:, :len(ix)] = k[b].T[:, ix]
        vc[:, :len(ix)] = v[b].T[:, ix]
        kt8.append(tile8(kc.astype(fp8), nk))
        vt.append(tile8t(vc.astype(bf16), nk))
        m = np.zeros((nk,), np.float32)
        m[:len(ix)] = 1.0
        kmv.append(np.ascontiguousarray(
            m.reshape(nk // P, P).T).astype(bf16))

    in_maps = []
    for c in range(NCORES):
        b, g = divmod(c, GROUPS)
        cols = slice(g * GW, (g + 1) * GW)
        in_maps.append({
            "qt8": qt8[b],
            "kt8": kt8[b],
            "vt": vt[b],
            "wq8": tile8((Wq[:, cols] * WSCL).astype(fp8), GW),
            "wk8": tile8((Wk[:, cols] * WSCL).astype(fp8), GW),
            "wv": tile8(Wv[:, cols].astype(bf16), GW),
            "km": kmv[b],
            "bq": (np.asarray(bq, np.float32)[cols] * WSCL).astype(bf16),
            "bk": (np.asarray(bk, np.float32)[cols] * WSCL).astype(bf16),
            "bv": np.asarray(bv, np.float32)[cols].astype(bf16),
        })
    return nk, use_bias, in_maps


def _run(nc, in_maps, trace=False):
    from concourse.bass_utils import run_bass_kernel_spmd

    return run_bass_kernel_spmd(
        nc, in_maps, core_ids=list(range(NCORES)), trace=trace
    )


def _assemble(results):
    out = np.empty((B, S, D), np.float32)
    for c in range(NCORES):
        b, g = divmod(c, GROUPS)
        out[b, :, g * GW:(g + 1) * GW] = results[c]["out"].reshape(S, GW).astype(np.float32)
    return out


def kernel(q, k, v, mask, Wq, bq, Wk, bk, Wv, bv):
    nk, use_bias, in_maps = _make_in_maps(q, k, v, mask, Wq, bq, Wk, bk,
                                          Wv, bv)
    res = _run(_get_nc(nk, use_bias), in_maps, trace=False)
    return _assemble(res.results)


def _install_ntff_hook():
    import sys
    import types

    import antenv

    if "antenv.axon_hooks" in sys.modules:
        return
    mod = types.ModuleType("antenv.axon_hooks")
    state = {"hook": None}
    mod.set_axon_ntff_profile_hook = lambda h: state.__setitem__("hook", h)
    mod.get_axon_ntff_profile_hook = lambda: state["hook"]
    sys.modules["antenv.axon_hooks"] = mod
    antenv.axon_hooks = mod
    try:
        from trn_agent_boot.trn_boot import _ntff_profile_via_ctypes

        mod.set_axon_ntff_profile_hook(
            _ntff_profile_via_ctypes("/opt/axon/libaxon_pjrt.so")
        )
    except Exception as e:
        print(f"ntff hook registration failed: {e}")


def kernel_traced(q, k, v, mask, Wq, bq, Wk, bk, Wv, bv):
    _install_ntff_hook()
    nk, use_bias, in_maps = _make_in_maps(q, k, v, mask, Wq, bq, Wk, bk,
                                          Wv, bv)
    res = _run(_get_nc(nk, use_bias), in_maps, trace=True)
    return _assemble(res.results), res.exec_time_ns
